# revision 1
# baseline (speedup 1.0000x reference)
"""Trainium2 Bass kernel for nn_ExpandEvecs.

Computes, for evecs [B=4, C=1, M=1024, K=32] and max_lvl=16, the stack of
cumulative low-rank reconstructions
    out[b, l] = V[:, :l+1] @ V[:, :l+1]^T      (V = evecs[b, 0, :, :max_lvl])
returned as [B, max_lvl, M, M] float32 (256 MiB) — a purely output-DMA-bound
problem (~32 MiB written per core across 8 cores).

Sharding: core i handles batch b = i//2 and row-half h = i%2 (512 rows of
every level's M x M matrix).

Precision trick: on the host each eigenvector value v is split as
v = H + E with H = fp16(v), E = fp16(v - H) (22 mantissa bits total). The
level-l Gram matrix is
    sum_{k<=l} v_k v_k^T ~= sum_{k<=l} (H_k H_k^T + H_k E_k^T + E_k H_k^T)
(the dropped E E^T term is ~2^-22 relative). With rows interleaved as
lhsT rows [H_k, E_k, H_k] and rhs rows [H_k, H_k, E_k], ONE fp16 matmul of
contraction 3(l+1) <= 48 computes all three terms, streaming at the full
1 col/cycle PE rate, accumulating exactly in fp32 PSUM.

Per core the kernel is then just: 2 tiny input DMAs; per level l, 8 matmuls
(lhsT = stride-4 column slices of vt3_rows so partition p carries rows
4p..4p+3, rhs = vt3_full 512-chunks) -> single-bank PSUM tiles; per-chunk
PSUM->SBUF copies alternating VectorE/ScalarE; 2 MiB per-level output DMAs
(16 KiB descriptors) alternating between the two HWDGE rings (sync/scalar),
with the first two levels DMA'd per 256 KiB chunk so output bandwidth ramps
immediately after the ~7 us framework preamble.

Measured: ~98-105 us per core HW exec (max across cores typically 105-115 us
depending on HBM-stack contention between paired cores), vs a ~94 us
per-core DMA roofline at ~360 GB/s HBM write bandwidth. Output matches the
fp32 reference to ~3e-7 scale-relative absmax.
"""

import sys

for _p in ("/root/.axon_site/_ro/trn_rl_repo", "/opt/trn_rl_repo"):
    if _p not in sys.path:
        sys.path.insert(0, _p)

import numpy as np

import concourse.bacc as bacc
import concourse.mybir as mybir
from concourse.tile import TileContext
from concourse import bass_utils

B, C, M, K, L = 4, 1, 1024, 32, 16
HALF = M // 2
P = 128
R3 = 3 * L  # 48 interleaved rows
F32 = mybir.dt.float32
F16 = mybir.dt.float16

OUT_BUFS = 5
FINE_LEVELS = 2


SPLIT_RINGS = False
SHORT_HIGH_ENGINES = False


def build_nc(out_bufs=OUT_BUFS, fine=2):
    nc = bacc.Bacc("TRN2", target_bir_lowering=False, debug=False)
    vt3_full = nc.dram_tensor("vt3_full", [R3, M], F16, kind="ExternalInput")
    vt3_rows = nc.dram_tensor("vt3_rows", [R3, HALF], F16, kind="ExternalInput")
    out = nc.dram_tensor("out", [L, HALF, M], F32, kind="ExternalOutput")

    # Partition p carries rows 4p..4p+3 of each level (g = row mod 4), so a
    # level's DMA sees 16 KiB contiguous DRAM per partition — the biggest
    # descriptors this layout allows.
    out_r = out.ap().rearrange("l (p g) n -> l p g n", g=4)

    with TileContext(nc) as tc:
        with (
            tc.tile_pool(name="consts", bufs=1) as consts,
            tc.tile_pool(name="outp", bufs=out_bufs) as outp,
            tc.tile_pool(name="psum", bufs=8, space="PSUM") as psump,
        ):
            # Split input DMAs: the first 6 interleaved rows cover levels
            # 0-1, so the first matmuls start as soon as the small prefix
            # lands instead of waiting for the full 48-row transfer.
            vt_r = consts.tile([R3, HALF], F16)
            nc.scalar.dma_start(out=vt_r[0:6, :], in_=vt3_rows.ap()[0:6])
            vt_f = consts.tile([R3, M], F16)
            nc.sync.dma_start(out=vt_f[0:6, :], in_=vt3_full.ap()[0:6])
            nc.scalar.dma_start(out=vt_r[6:R3, :], in_=vt3_rows.ap()[6:R3])
            nc.sync.dma_start(out=vt_f[6:R3, :], in_=vt3_full.ap()[6:R3])

            # lhsT for row-slot g selects every 4th eigenvector column so the
            # matmul writes row 4p+g on partition p.
            vt_r4 = vt_r[:, :].rearrange("k (p g) -> k g p", g=4)

            # Every 512-wide chunk gets its own single-bank PSUM tile so the
            # PE streams ahead without zero-region WAR stalls; per-chunk
            # copies (alternating VectorE/ScalarE) assemble per-level SBUF
            # tiles. First levels DMA per 256 KiB chunk so output bandwidth
            # ramps immediately; later levels DMA 2 MiB per level.
            FINE = fine
            cnt = 0
            for l in range(L):
                r = 3 * (l + 1)
                ot = outp.tile([P, 4096], F32)
                for j in range(8):
                    g = j // 2
                    nch = j % 2
                    pt = psump.tile([P, 512], F32)
                    nc.tensor.matmul(
                        pt,
                        vt_r4[0:r, g, :],
                        vt_f[0:r, nch * 512 : (nch + 1) * 512],
                        start=True,
                        stop=True,
                    )
                    dst = ot[:, j * 512 : (j + 1) * 512]
                    if cnt % 2 == 0:
                        nc.vector.tensor_copy(out=dst, in_=pt)
                    else:
                        nc.scalar.copy(out=dst, in_=pt)
                    if l < FINE and nch == 1:
                        dma_eng = nc.sync if cnt % 2 == 0 else nc.scalar
                        dma_eng.dma_start(
                            out=out_r[l][:, g : g + 1, :],
                            in_=ot[:, g * M : (g + 1) * M].rearrange(
                                "p (g n) -> p g n", g=1
                            ),
                        )
                    cnt += 1
                if l >= FINE:
                    if SHORT_HIGH_ENGINES:
                        dma_eng = nc.sync if l % 2 == 0 else nc.scalar
                        # 120 x 16 KiB descriptors: HWDGE round-robins
                        # descriptors from engine 0 per DMA, so engines 8-15
                        # get 7 descriptors vs 8 — sheds ~11% load off the
                        # most-often-degraded high engine indices.
                        dma_eng.dma_start(
                            out=out_r[l][0:120],
                            in_=ot[0:120, :].rearrange("p (g n) -> p g n", n=M),
                        )
                        # partitions 120-127 ride as 64 x 2 KiB descriptors
                        # (+1 small desc on every engine's queue).
                        dma_eng.dma_start(
                            out=out_r[l][120:P].rearrange("p g (c n) -> p g c n", c=2),
                            in_=ot[120:P, :].rearrange("p (g c n) -> p g c n", g=4, c=2),
                        )
                    elif SPLIT_RINGS:
                        nc.sync.dma_start(
                            out=out_r[l][0:64],
                            in_=ot[0:64, :].rearrange("p (g n) -> p g n", n=M),
                        )
                        nc.scalar.dma_start(
                            out=out_r[l][64:P],
                            in_=ot[64:P, :].rearrange("p (g n) -> p g n", n=M),
                        )
                    elif l < FINE + 2:
                        for hg in range(2):
                            dma_eng = nc.sync if (l + hg) % 2 == 0 else nc.scalar
                            dma_eng.dma_start(
                                out=out_r[l][:, hg * 2 : (hg + 1) * 2, :],
                                in_=ot[:, hg * 2 * M : (hg + 1) * 2 * M].rearrange(
                                    "p (g n) -> p g n", n=M
                                ),
                            )
                    else:
                        dma_eng = nc.sync if l % 2 == 0 else nc.scalar
                        dma_eng.dma_start(
                            out=out_r[l],
                            in_=ot[:, :].rearrange("p (g n) -> p g n", n=M),
                        )
    nc.compile()
    return nc


_NC_CACHE = {}


def _get_nc():
    key = (OUT_BUFS, FINE_LEVELS)
    if key not in _NC_CACHE:
        _NC_CACHE[key] = build_nc(OUT_BUFS, FINE_LEVELS)
    return _NC_CACHE[key]


def _interleave3(a, b, c):
    """rows [a0,b0,c0,a1,b1,c1,...] from [L, N] each -> [3L, N]."""
    out = np.empty((3 * a.shape[0], a.shape[1]), dtype=a.dtype)
    out[0::3] = a
    out[1::3] = b
    out[2::3] = c
    return out


def make_in_maps(evecs):
    evecs = np.asarray(evecs, dtype=np.float32)
    in_maps = []
    for core in range(8):
        b, h = core // 2, core % 2
        vt = np.ascontiguousarray(evecs[b, 0, :, :L].T)  # [L, M] fp32
        hi = vt.astype(np.float16)
        lo = (vt - hi.astype(np.float32)).astype(np.float16)
        full = _interleave3(hi, hi, lo)  # rhs rows: [H, H, E]
        hr = hi[:, h * HALF : (h + 1) * HALF]
        lr = lo[:, h * HALF : (h + 1) * HALF]
        rows = _interleave3(hr, lr, hr)  # lhsT rows: [H, E, H]
        in_maps.append(
            {
                "vt3_full": np.ascontiguousarray(full),
                "vt3_rows": np.ascontiguousarray(rows),
            }
        )
    return in_maps


def assemble(results):
    full = np.empty((B, L * C, M, M), dtype=np.float32)
    for core in range(8):
        b, h = core // 2, core % 2
        full[b, :, h * HALF : (h + 1) * HALF, :] = results[core]["out"]
    return full


def kernel(evecs, max_lvl):
    assert int(max_lvl) == L, f"kernel hardcodes max_lvl={L}, got {max_lvl}"
    nc = _get_nc()
    res = bass_utils.run_bass_kernel_spmd(nc, make_in_maps(evecs), list(range(8)))
    return assemble(res.results)

